# revision 23
# baseline (speedup 1.0000x reference)
"""Trainium2 Bass kernel: AtomEmbeddingAndSumLastLayer (segment_reduce).

Computes: out = normalize(relu(segment_sum(relu(x @ W.T + b), batch)))
  x [1M, 118] f32, W [64, 118], b [64], batch [1M] sorted int in [0, 4096).

Strategy (8 NeuronCores, no collectives needed):
  - Atoms are cut at segment-aligned boundaries on the host so core c owns
    exactly segments [512c, 512(c+1)); per-core outputs concatenate.
  - Host pre-transposes x to xT [128, A] fp8-e4m3 with a ones-row at 118
    (folds the bias into the matmul) and zero rows above; atoms are grouped
    into 4 "superwindows" of 128 segments, each made of 4 windows of 32
    segments whose 128-atom tiles are interleaved quad-wise.
  - Device, per 128-atom tile:
      h_psum[128, 64] = xT_tile.T @ WT            (TensorE, fp8 x bf16)
      h_sb = relu(h_psum) -> bf16                 (ScalarE, chunked)
      oh[128, 32] = (iota == seg_local)           (VectorE, one op/superwin)
      s_psum[32q:32q+32, 64] += oh.T @ h_sb       (TensorE col-group q —
                                                   4 windows' seg-matmuls run
                                                   on disjoint 32-col strips)
    Epilogue per superwindow on [128, 64]: rowwise max, recip, scale, DMA.
"""

import os
import sys
import numpy as np

sys.path.insert(0, "/opt/trn_rl_repo")

import ml_dtypes  # noqa: E402

N_ATOMS = 1_000_000
D_IN = 118
K_DIM = 128  # 118 features + ones-row (bias) at 118, zero-padded to 128
ONES_ROW = D_IN
D_OUT = 64
NUM_SEG = 4096
N_CORES = 8
SEGS_PER_CORE = NUM_SEG // N_CORES  # 512
G_W = 32  # segments per window (one PE col-group)
QUAD = 4  # windows per superwindow (= PE col-groups used)
SUPER = SEGS_PER_CORE // (G_W * QUAD)  # 4 superwindows per core
P = 128
CHUNK = 16  # tiles per compute chunk (= 4 quads; 16*64 f32 = 2 psum banks)
RELU_MOD = 0  # every RELU_MOD-th relu chunk on VectorE (0 = all ScalarE)
XBUFS = 6
HBUFS = 4
OHBUFS = 3
PAD_ID = 200.0  # local seg id for padding atoms; never matches iota [0, G_W)

BF16 = ml_dtypes.bfloat16
FP8 = ml_dtypes.float8_e4m3

_CACHE = {}


def _build_graph(t_q: int, postprocess: bool = True):
    """Build the SPMD Bass graph for one core.

    t_q = padded tiles per window (multiple of QUAD); each superwindow has
    QUAD * t_q interleaved tiles.
    """
    import concourse.bass as bass
    import concourse.tile as tile
    from concourse import mybir
    from contextlib import ExitStack

    sw_tiles = QUAD * t_q  # tiles per superwindow
    n_tiles = SUPER * sw_tiles
    a_cols = n_tiles * P

    nc = bass.Bass(target_bir_lowering=False)

    xt = nc.declare_dram_parameter("xt", [K_DIM, a_cols], mybir.dt.float8e4, False)
    seg = nc.declare_dram_parameter("seg", [P, n_tiles], mybir.dt.bfloat16, False)
    wt = nc.declare_dram_parameter("wt", [K_DIM, D_OUT], mybir.dt.bfloat16, False)
    iota = nc.declare_dram_parameter("iota", [P, G_W], mybir.dt.bfloat16, False)
    out = nc.declare_dram_parameter(
        "out", [SEGS_PER_CORE, D_OUT], mybir.dt.float32, True
    )

    with ExitStack() as ctx:
        tc = ctx.enter_context(tile.TileContext(nc))
        consts = ctx.enter_context(tc.tile_pool(name="consts", bufs=1))
        xpool = ctx.enter_context(tc.tile_pool(name="xp", bufs=XBUFS))
        hpool = ctx.enter_context(tc.tile_pool(name="hp", bufs=HBUFS))
        ohpool = ctx.enter_context(tc.tile_pool(name="ohp", bufs=OHBUFS))
        psum_h = ctx.enter_context(tc.tile_pool(name="psh", bufs=3, space="PSUM"))
        psum_s = ctx.enter_context(tc.tile_pool(name="pss", bufs=2, space="PSUM"))
        epi = ctx.enter_context(tc.tile_pool(name="epi", bufs=2))

        wt_sb = consts.tile([K_DIM, D_OUT], mybir.dt.bfloat16)
        nc.sync.dma_start(out=wt_sb[:], in_=wt[:, :])
        iota_sb = consts.tile([P, G_W], mybir.dt.bfloat16)
        nc.sync.dma_start(out=iota_sb[:], in_=iota[:, :])
        seg_sb = consts.tile([P, n_tiles], mybir.dt.bfloat16)
        nc.sync.dma_start(out=seg_sb[:], in_=seg[:, :])

        # "touch" the consts on VectorE once so later ops don't each carry
        # multiple DMA-lane semaphore waits (walrus wait-slot limit).
        dummy_a = consts.tile([P, 1], mybir.dt.bfloat16)
        nc.vector.tensor_copy(out=dummy_a[:], in_=iota_sb[:, :1])
        dummy_b = consts.tile([P, 1], mybir.dt.bfloat16)
        nc.vector.tensor_copy(out=dummy_b[:], in_=seg_sb[:, :1])
        dummy_c = consts.tile([K_DIM, 1], mybir.dt.bfloat16)
        nc.vector.tensor_copy(out=dummy_c[:], in_=wt_sb[:, :1])
        zeros_sb = consts.tile([P, P], mybir.dt.bfloat16)
        nc.vector.memset(zeros_sb[:], 0.0)
        # prewarm ScalarE's activation table during the initial x DMA
        dummy_d = consts.tile([P, 1], mybir.dt.bfloat16)
        nc.scalar.activation(
            out=dummy_d[:], in_=dummy_a[:],
            func=mybir.ActivationFunctionType.Relu,
        )

        n_chunks = sw_tiles // CHUNK
        for sw in range(SUPER):
            base_t = sw * sw_tiles
            n_pieces = 4 if sw == 0 else 2
            piece = sw_tiles * P // n_pieces
            x_pieces = []
            for pi in range(n_pieces):
                xp_t = xpool.tile([K_DIM, piece], mybir.dt.float8e4,
                                  tag=f"xh{n_pieces}")
                p0 = base_t * P + pi * piece
                nc.sync.dma_start(out=xp_t[:], in_=xt[:, p0 : p0 + piece])
                x_pieces.append(xp_t)
            s_ps = psum_s.tile([P, D_OUT], mybir.dt.float32)
            # open the accumulation group over the whole bank with a zero
            # matmul (clears has_written for all 128 partitions at once);
            # the quad col-group seg-matmuls then accumulate with start=False
            nc.tensor.matmul(
                out=s_ps[:],
                lhsT=zeros_sb[:],
                rhs=wt_sb[:],
                start=True,
                stop=False,
                skip_group_check=True,
            )
            # whole superwindow's one-hot in one DVE op:
            # oh[p, m, g] = (iota[p, g] == seg[p, base_t + m])
            oh_win = ohpool.tile([P, sw_tiles * G_W], mybir.dt.bfloat16)
            iota_ap = iota_sb[:]
            in0 = bass.AP(
                tensor=iota_ap.tensor, offset=iota_ap.offset,
                ap=[iota_ap.ap[0], [0, sw_tiles], iota_ap.ap[1]],
            )
            seg_sl = seg_sb[:, base_t : base_t + sw_tiles]
            in1 = bass.AP(
                tensor=seg_sl.tensor, offset=seg_sl.offset,
                ap=[seg_sl.ap[0], seg_sl.ap[1], [0, G_W]],
            )
            nc.vector.tensor_tensor(
                out=oh_win[:].rearrange("p (t g) -> p t g", g=G_W),
                in0=in0, in1=in1, op=mybir.AluOpType.is_equal,
            )
            for chv in range(n_chunks):
                h_ps = psum_h.tile([P, CHUNK * D_OUT], mybir.dt.float32)
                for i in range(CHUNK):
                    t = chv * CHUNK + i
                    pi = (t * P) // piece
                    toff = pi * piece
                    nc.tensor.matmul(
                        out=h_ps[:, i * D_OUT : (i + 1) * D_OUT],
                        lhsT=x_pieces[pi][:, t * P - toff : (t + 1) * P - toff],
                        rhs=wt_sb[:],
                        start=True,
                        stop=True,
                    )
                h_sb = hpool.tile([P, CHUNK * D_OUT], mybir.dt.bfloat16)
                if RELU_MOD and (sw * n_chunks + chv) % RELU_MOD == RELU_MOD - 1:
                    nc.vector.tensor_scalar_max(
                        out=h_sb[:], in0=h_ps[:], scalar1=0.0
                    )
                else:
                    nc.scalar.activation(
                        out=h_sb[:],
                        in_=h_ps[:],
                        func=mybir.ActivationFunctionType.Relu,
                    )
                # seg-matmuls: window q of the quad accumulates on PE
                # col-group q into psum partitions [32q, 32q+32)
                for i in range(CHUNK):
                    t = chv * CHUNK + i
                    q = i % QUAD
                    nc.tensor.matmul(
                        out=s_ps[G_W * q : G_W * (q + 1), :],
                        lhsT=oh_win[:, t * G_W : (t + 1) * G_W],
                        rhs=h_sb[:, i * D_OUT : (i + 1) * D_OUT],
                        start=False,
                        stop=(chv == n_chunks - 1 and i == CHUNK - 1),
                        tile_position=(0, G_W * q),
                        skip_group_check=True,
                    )
            # epilogue: max-normalize the superwindow's 128 segment rows
            s_sb = epi.tile([P, D_OUT], mybir.dt.float32)
            nc.vector.tensor_copy(out=s_sb[:], in_=s_ps[:])
            mx = epi.tile([P, 1], mybir.dt.float32)
            nc.vector.tensor_reduce(
                out=mx[:], in_=s_sb[:], axis=mybir.AxisListType.X,
                op=mybir.AluOpType.max,
            )
            rc = epi.tile([P, 1], mybir.dt.float32)
            nc.vector.reciprocal(out=rc[:], in_=mx[:])
            o_sb = epi.tile([P, D_OUT], mybir.dt.float32)
            nc.vector.tensor_scalar_mul(out=o_sb[:], in0=s_sb[:], scalar1=rc[:])
            nc.sync.dma_start(
                out=out[sw * P : (sw + 1) * P, :], in_=o_sb[:]
            )

    if postprocess:
        _split_multi_waits(nc)
    return nc


def _split_multi_waits(nc):
    """walrus allows a single embedded sync wait per compute instruction.
    Move extra waits onto same-engine NoOps inserted just before."""
    from concourse import mybir

    n = 0
    for f in nc.m.functions:
        for blk in f.blocks:
            new_insts = []
            for inst in blk.instructions:
                si = getattr(inst, "sync_info", None)
                if si is not None and si.on_wait and len(si.on_wait) > 1:
                    extras, keep = si.on_wait[:-1], si.on_wait[-1:]
                    for wsub in extras:
                        nop = mybir.InstNoOp(
                            name=f"{inst.name}_waitnop{n}",
                            sync_info=mybir.SyncInfo(on_wait=[wsub], on_update=[]),
                            bass_nofuse=True,
                            engine=inst.engine,
                        )
                        n += 1
                        new_insts.append(nop)
                    si.on_wait = keep
                new_insts.append(inst)
            blk.instructions[:] = new_insts


def _prepare_inputs(x, w_mat, b, batch):
    """Host-side sharding/layout. Returns (in_maps, t_q)."""
    x = np.asarray(x, dtype=np.float32)
    w_mat = np.asarray(w_mat, dtype=np.float32)
    b = np.asarray(b, dtype=np.float32)
    batch = np.asarray(batch).astype(np.int64)

    # window boundaries: window j (global, 32 segs) holds atoms [wb[j], wb[j+1])
    wb = np.searchsorted(batch, np.arange(0, NUM_SEG + 1, G_W))
    counts = np.diff(wb)
    t_q = int(np.ceil(counts.max() / P))
    t_q = ((t_q + QUAD - 1) // QUAD) * QUAD  # multiple of QUAD

    sw_tiles = QUAD * t_q
    n_tiles = SUPER * sw_tiles
    a_cols = n_tiles * P

    wt = np.zeros((K_DIM, D_OUT), dtype=BF16)
    wt[:D_IN] = w_mat.T.astype(BF16)
    wt[ONES_ROW] = b.astype(BF16)
    iota = np.broadcast_to(
        np.arange(G_W, dtype=np.float32), (P, G_W)
    ).astype(BF16)

    xb = x.astype(FP8)
    n_win_per_core = SEGS_PER_CORE // G_W  # 16
    in_maps = []
    for c in range(N_CORES):
        xt_c = np.zeros((K_DIM, a_cols), dtype=FP8)
        seg_c = np.full((n_tiles, P), PAD_ID, dtype=np.float32)
        for sw in range(SUPER):
            for q in range(QUAD):
                gw = c * n_win_per_core + sw * QUAD + q  # global window id
                a0, a1 = wb[gw], wb[gw + 1]
                cnt = a1 - a0
                loc = (batch[a0:a1] - gw * G_W).astype(np.float32)
                # tile k of this window sits at interleaved slot (k*QUAD + q)
                for k in range((cnt + P - 1) // P):
                    m = sw * sw_tiles + k * QUAD + q  # global tile index
                    s0, s1 = k * P, min((k + 1) * P, cnt)
                    nseg = s1 - s0
                    col0 = m * P
                    xt_c[:D_IN, col0 : col0 + nseg] = xb[a0 + s0 : a0 + s1].T
                    xt_c[ONES_ROW, col0 : col0 + nseg] = 1.0
                    seg_c[m, :nseg] = loc[s0:s1]
        seg_c = np.ascontiguousarray(seg_c.T).astype(BF16)
        in_maps.append({"xt": xt_c, "seg": seg_c, "wt": wt, "iota": iota})
    return in_maps, t_q


def _install_ntff_hook_shim():
    """The trimmed container's antenv lacks axon_hooks; recreate it so
    run_bass_kernel_spmd(trace=True) can profile via the axon .so."""
    import types

    if "antenv.axon_hooks" in sys.modules:
        return
    try:
        from trn_agent_boot.trn_boot import _ntff_profile_via_ctypes

        hook = _ntff_profile_via_ctypes("/opt/axon/libaxon_pjrt.so")
    except Exception:
        hook = None
    mod = types.ModuleType("antenv.axon_hooks")
    mod._hook = hook
    mod.get_axon_ntff_profile_hook = lambda: mod._hook
    mod.set_axon_ntff_profile_hook = lambda h: setattr(mod, "_hook", h)
    sys.modules["antenv.axon_hooks"] = mod


def kernel(x, W, b, batch, num_segments):
    from concourse.bass_utils import run_bass_kernel_spmd

    assert int(num_segments) == NUM_SEG
    in_maps, t_q = _prepare_inputs(x, W, b, batch)

    key = (t_q, G_W, QUAD, CHUNK, RELU_MOD, XBUFS, HBUFS, OHBUFS)
    if key not in _CACHE:
        _CACHE[key] = _build_graph(t_q)
    nc = _CACHE[key]

    trace = bool(int(os.environ.get("KERNEL_TRACE", "0")))
    if trace:
        _install_ntff_hook_shim()
    res = run_bass_kernel_spmd(
        nc, in_maps, core_ids=list(range(N_CORES)), trace=trace
    )
    kernel.last_result = res
    out = np.concatenate([r["out"] for r in res.results], axis=0)
    return out.astype(np.float32)


kernel.last_result = None
